# revision 5
# baseline (speedup 1.0000x reference)
"""Trainium2 Bass kernel for nn_CRFTModule (moe_routing).

Pure data parallel over batch: 8 cores, one batch row (4096 tokens) each.

Math per core (batch b, S=4096 tokens, H=1024):
  z      = gelu(x @ W1 + b1) @ W2 + b2              (critical-path detector)
  mask   = z > logit(0.7)                            (compare in logit space)
  aw     = softmax(x[last] @ sel_w + sel_b)          (adapter selector, 4-way)
  t      = gelu(x @ Dcat + db)                       (all 4 down-projs, [S,32])
  wm     = 0.3 * mask * (sum_a aw[a] (t_a @ up_w[a] + up_b[a]))
  out    = x + wm

Device pipeline (f16 matmuls, fp32 psum accumulation, fp32 residual):
  - x loaded naturally [tok, H], cast to f16 (DVE), transposed on PE
    (128x128 blocks) into XT [H-part, tok] for the H-contraction matmuls.
  - detector mm1/mm2, down-proj and the mask broadcast run on PE in f16.
  - adapter weighting folded into a K=36 up-matmul whose output is NATURAL
    layout [tok, H]; the residual add reads exact fp32 x.
  - softmax computed with the tanh identity (exp(z)=(1+tanh(z/2))/(1-tanh(z/2)))
    so the whole kernel uses one ACT table set (gelu_and_others).
"""
import math

import numpy as np

import concourse.bacc as bacc
import concourse.mybir as mybir
from concourse.bass import ts
from concourse.tile import TileContext
from concourse.bass_utils import run_bass_kernel_spmd

dt = mybir.dt
AF = mybir.ActivationFunctionType
ALU = mybir.AluOpType

B, S, H = 8, 4096, 1024
A_DIM, N_ADAPT = 8, 4
PD = H // 2              # 512 detector hidden dim
SCALE = 0.3
THRESH = 0.7
N_CORES = 8
T = 512                  # tokens per tile
N_TILES = S // T         # 8
KUP = N_ADAPT * A_DIM + N_ADAPT  # 36


def _build():
    nc = bacc.Bacc("TRN2", target_bir_lowering=False, debug=False)

    x = nc.declare_dram_parameter("x", [S, H], dt.float32, isOutput=False)
    w1s = nc.declare_dram_parameter("w1s", [128, 8 * PD], dt.float16, isOutput=False)
    dcats = nc.declare_dram_parameter("dcats", [128, 8 * 32], dt.float16, isOutput=False)
    w2s = nc.declare_dram_parameter("w2s", [128, 4], dt.float16, isOutput=False)
    b1s = nc.declare_dram_parameter("b1s", [128, 4], dt.float32, isOutput=False)
    dbs = nc.declare_dram_parameter("dbs", [32, 1], dt.float32, isOutput=False)
    u36 = nc.declare_dram_parameter("u36", [KUP, H], dt.float16, isOutput=False)
    selws = nc.declare_dram_parameter("selws", [128, 8 * 4], dt.float32, isOutput=False)
    selbh = nc.declare_dram_parameter("selbh", [4, 1], dt.float32, isOutput=False)
    e36 = nc.declare_dram_parameter("e36", [4, KUP], dt.float32, isOutput=False)
    i128h = nc.declare_dram_parameter("i128h", [128, 128], dt.float16, isOutput=False)
    i4 = nc.declare_dram_parameter("i4", [4, 4], dt.float32, isOutput=False)
    ones14 = nc.declare_dram_parameter("ones14", [1, 4], dt.float32, isOutput=False)
    ones36 = nc.declare_dram_parameter("ones36", [1, KUP], dt.float16, isOutput=False)
    thr = nc.declare_dram_parameter("thr", [1, 1], dt.float32, isOutput=False)

    out = nc.declare_dram_parameter("out", [S, H], dt.float32, isOutput=True)

    with TileContext(nc) as tc:
        with (
            tc.tile_pool(name="consts", bufs=1) as cp,
            tc.tile_pool(name="work", bufs=2) as wp,
            tc.tile_pool(name="psum", bufs=2, space="PSUM") as pp,
        ):
            # ---- constants into SBUF ----
            w1_sb = cp.tile([128, 8, PD], dt.float16, tag="w1")
            nc.sync.dma_start(out=w1_sb[:], in_=w1s.rearrange("p (c n) -> p c n", c=8))
            d_sb = cp.tile([128, 8, 32], dt.float16, tag="dcat")
            nc.sync.dma_start(out=d_sb[:], in_=dcats.rearrange("p (c n) -> p c n", c=8))
            w2_sb = cp.tile([128, 4], dt.float16, tag="w2")
            nc.sync.dma_start(out=w2_sb[:], in_=w2s[:])
            b1_sb = cp.tile([128, 4], dt.float32, tag="b1")
            nc.sync.dma_start(out=b1_sb[:], in_=b1s[:])
            db_sb = cp.tile([32, 1], dt.float32, tag="db")
            nc.sync.dma_start(out=db_sb[:], in_=dbs[:])
            u_sb = cp.tile([KUP, H], dt.float16, tag="u36")
            nc.sync.dma_start(out=u_sb[:], in_=u36[:])
            selw_sb = cp.tile([128, 8, 4], dt.float32, tag="selw")
            nc.sync.dma_start(out=selw_sb[:], in_=selws.rearrange("p (c a) -> p c a", c=8))
            selb_sb = cp.tile([4, 1], dt.float32, tag="selb")
            nc.sync.dma_start(out=selb_sb[:], in_=selbh[:])
            e36_sb = cp.tile([4, KUP], dt.float32, tag="e36")
            nc.sync.dma_start(out=e36_sb[:], in_=e36[:])
            idh_sb = cp.tile([128, 128], dt.float16, tag="idh")
            nc.sync.dma_start(out=idh_sb[:], in_=i128h[:])
            i4_sb = cp.tile([4, 4], dt.float32, tag="i4")
            nc.sync.dma_start(out=i4_sb[:], in_=i4[:])
            o14_sb = cp.tile([1, 4], dt.float32, tag="o14")
            nc.sync.dma_start(out=o14_sb[:], in_=ones14[:])
            o36_sb = cp.tile([1, KUP], dt.float16, tag="o36")
            nc.sync.dma_start(out=o36_sb[:], in_=ones36[:])
            thr_sb = cp.tile([1, 1], dt.float32, tag="thr")
            nc.sync.dma_start(out=thr_sb[:], in_=thr[:])

            # ---- adapter selector (once per core) ----
            # last token, transposed: [128, 8] (col c = H chunk c)
            xlast = cp.tile([128, 8], dt.float32, tag="xlast")
            nc.sync.dma_start(
                out=xlast[:],
                in_=x.rearrange("s (c p) -> p s c", p=128)[:, S - 1, :],
            )
            ps_sel = pp.tile([4, 1], dt.float32, tag="small")
            for c in range(8):
                nc.tensor.matmul(
                    ps_sel[:], selw_sb[:, c, :], xlast[:, c : c + 1],
                    start=(c == 0), stop=(c == 7),
                )
            # t = tanh((z + sel_b)/2)  -> exp(z+sel_b) = (1+t)/(1-t)
            t4 = cp.tile([4, 1], dt.float32, tag="t4")
            nc.scalar.activation(t4[:], ps_sel[:], AF.Tanh, bias=selb_sb[:], scale=0.5)
            num4 = cp.tile([4, 1], dt.float32, tag="num4")
            nc.vector.tensor_scalar(num4[:], t4[:], 1.0, None, ALU.add)
            den4 = cp.tile([4, 1], dt.float32, tag="den4")
            nc.vector.tensor_scalar(den4[:], t4[:], -1.0, 1.0, ALU.mult, ALU.add)
            rden4 = cp.tile([4, 1], dt.float32, tag="rden4")
            nc.vector.reciprocal(rden4[:], den4[:])
            e4 = cp.tile([4, 1], dt.float32, tag="e4")
            nc.vector.tensor_mul(e4[:], num4[:], rden4[:])
            # transpose e4 -> [1,4], sum, reciprocal, broadcast back
            ps_et = pp.tile([1, 4], dt.float32, tag="small")
            nc.tensor.matmul(ps_et[:], e4[:], i4_sb[:], start=True, stop=True)
            ssum = cp.tile([1, 1], dt.float32, tag="ssum")
            nc.vector.reduce_sum(ssum[:], ps_et[:], axis=mybir.AxisListType.X)
            rsum = cp.tile([1, 1], dt.float32, tag="rsum")
            nc.vector.reciprocal(rsum[:], ssum[:])
            ps_rs = pp.tile([4, 1], dt.float32, tag="small")
            nc.tensor.matmul(ps_rs[:], o14_sb[:], rsum[:], start=True, stop=True)
            w4 = cp.tile([4, 1], dt.float32, tag="w4")
            nc.vector.tensor_tensor(w4[:], e4[:], ps_rs[:], ALU.mult)
            # expand to wvec36 [36,1]
            ps_wv = pp.tile([KUP, 1], dt.float32, tag="small")
            nc.tensor.matmul(ps_wv[:], e36_sb[:], w4[:], start=True, stop=True)
            wv_sb = cp.tile([KUP, 1], dt.float32, tag="wv")
            nc.scalar.copy(wv_sb[:], ps_wv[:])

            # ---- main loop over token tiles ----
            for i in range(N_TILES):
                X = [
                    wp.tile([128, H], dt.float32, tag="X", name=f"X{i}_{j}", bufs=8)
                    for j in range(4)
                ]
                for j in range(4):
                    nc.sync.dma_start(
                        out=X[j][:], in_=x[i * T + j * 128 : i * T + (j + 1) * 128, :]
                    )
                Xh = [
                    wp.tile([128, H], dt.float16, tag="Xh", name=f"Xh{i}_{j}", bufs=6)
                    for j in range(4)
                ]
                for j in range(4):
                    nc.vector.tensor_copy(Xh[j][:], X[j][:])

                XT = []
                for c in range(8):
                    ps_xt = pp.tile([128, T], dt.float16, tag="xt", name=f"psxt{i}_{c}")
                    for j in range(4):
                        nc.tensor.transpose(
                            ps_xt[:, ts(j, 128)], Xh[j][:, ts(c, 128)], idh_sb[:]
                        )
                    xt = wp.tile([128, T], dt.float16, tag="XT", name=f"XT{i}_{c}", bufs=12)
                    nc.scalar.copy(xt[:], ps_xt[:])
                    XT.append(xt)

                # detector mm1 + gelu
                Hs = []
                for m in range(4):
                    ps_h = pp.tile([128, T], dt.float32, tag="h", name=f"psh{i}_{m}")
                    for c in range(8):
                        nc.tensor.matmul(
                            ps_h[:], w1_sb[:, c, ts(m, 128)], XT[c][:],
                            start=(c == 0), stop=(c == 7),
                        )
                    hm = wp.tile([128, T], dt.float16, tag="Hs", name=f"Hs{i}_{m}", bufs=5)
                    nc.scalar.activation(
                        hm[:], ps_h[:], AF.Gelu, bias=b1_sb[:, m : m + 1]
                    )
                    Hs.append(hm)

                # detector mm2 -> logits -> mask
                ps_p = pp.tile([1, T], dt.float32, tag="small")
                for m in range(4):
                    nc.tensor.matmul(
                        ps_p[:], w2_sb[:, m : m + 1], Hs[m][:],
                        start=(m == 0), stop=(m == 3),
                    )
                mask01 = wp.tile([1, T], dt.float16, tag="mask")
                nc.vector.tensor_scalar(mask01[:], ps_p[:], thr_sb[:], None, ALU.is_gt)

                # down-proj + gelu
                ps_t = pp.tile([32, T], dt.float32, tag="small")
                for c in range(8):
                    nc.tensor.matmul(
                        ps_t[:], d_sb[:, c, :], XT[c][:],
                        start=(c == 0), stop=(c == 7),
                    )
                G = wp.tile([32, T], dt.float16, tag="G")
                nc.scalar.activation(G[:], ps_t[:], AF.Gelu, bias=db_sb[:])

                # mask broadcast to 36 partitions; scale by adapter weights
                ps_m36 = pp.tile([KUP, T], dt.float32, tag="small")
                nc.tensor.matmul(ps_m36[:], o36_sb[:], mask01[:], start=True, stop=True)
                mw = wp.tile([KUP, T], dt.float16, tag="mw")
                nc.vector.tensor_scalar(mw[:], ps_m36[:], wv_sb[:], None, ALU.mult)
                nc.vector.tensor_tensor(mw[0:32, :], mw[0:32, :], G[:], ALU.mult)

                # up-proj (natural layout) + residual add + store
                for j in range(4):
                    for n in range(2):
                        ps_w = pp.tile([128, PD], dt.float32, tag="w", name=f"psw{i}_{j}_{n}")
                        nc.tensor.matmul(
                            ps_w[:], mw[:, ts(j, 128)], u_sb[:, ts(n, PD)],
                            start=True, stop=True,
                        )
                        nc.vector.tensor_tensor(
                            X[j][:, ts(n, PD)], X[j][:, ts(n, PD)], ps_w[:], ALU.add
                        )
                    nc.gpsimd.dma_start(
                        out=out[i * T + j * 128 : i * T + (j + 1) * 128, :],
                        in_=X[j][:],
                    )

    nc.compile()
    return nc


_CACHE = {}


def _get_nc():
    if "nc" not in _CACHE:
        _CACHE["nc"] = _build()
    return _CACHE["nc"]


def _host_params(inputs):
    f32 = np.float32
    f16 = np.float16
    pd_w1 = np.asarray(inputs["pd_w1"], f32)          # [H, PD]
    pd_b1 = np.asarray(inputs["pd_b1"], f32)          # [PD]
    pd_w2 = np.asarray(inputs["pd_w2"], f32)          # [PD, 1]
    pd_b2 = np.asarray(inputs["pd_b2"], f32)          # [1]
    down_w = np.asarray(inputs["down_w"], f32)        # [A, H, d]
    down_b = np.asarray(inputs["down_b"], f32)        # [A, d]
    up_w = np.asarray(inputs["up_w"], f32)            # [A, d, H]
    up_b = np.asarray(inputs["up_b"], f32)            # [A, H]
    sel_w = np.asarray(inputs["sel_w"], f32)          # [H, A]
    sel_b = np.asarray(inputs["sel_b"], f32)          # [A]

    # w1s[p, c*PD + n] = W1[c*128+p, n]
    w1s = np.ascontiguousarray(
        pd_w1.reshape(8, 128, PD).transpose(1, 0, 2).reshape(128, 8 * PD)
    ).astype(f16)
    # dcat[h, a*8+d] = down_w[a, h, d]
    dcat = down_w.transpose(1, 0, 2).reshape(H, 32)
    dcats = np.ascontiguousarray(
        dcat.reshape(8, 128, 32).transpose(1, 0, 2).reshape(128, 8 * 32)
    ).astype(f16)
    w2s = np.ascontiguousarray(
        pd_w2.reshape(4, 128).T
    ).astype(f16)                                      # [128, 4]
    b1s = np.ascontiguousarray(pd_b1.reshape(4, 128).T).astype(f32)  # [128,4]
    dbs = down_b.reshape(32, 1).astype(f32)
    # u36 rows 0..31 = 0.3*up_w[a][d,:], rows 32..35 = 0.3*up_b[a]
    u36 = np.concatenate(
        [SCALE * up_w.reshape(32, H), SCALE * up_b], axis=0
    ).astype(f16)                                      # [36, H]
    selws = np.ascontiguousarray(
        sel_w.reshape(8, 128, 4).transpose(1, 0, 2).reshape(128, 32)
    ).astype(f32)
    selbh = (sel_b / 2.0).reshape(4, 1).astype(f32)
    e36 = np.zeros((4, KUP), f32)
    for r in range(32):
        e36[r // 8, r] = 1.0
    for a in range(4):
        e36[a, 32 + a] = 1.0
    i128h = np.eye(128, dtype=f16)
    i4 = np.eye(4, dtype=f32)
    ones14 = np.ones((1, 4), f32)
    ones36 = np.ones((1, KUP), f16)
    thr = np.array(
        [[math.log(THRESH / (1.0 - THRESH)) - float(pd_b2[0])]], f32
    )
    return dict(
        w1s=w1s, dcats=dcats, w2s=w2s, b1s=b1s, dbs=dbs, u36=u36,
        selws=selws, selbh=selbh, e36=e36, i128h=i128h, i4=i4,
        ones14=ones14, ones36=ones36, thr=thr,
    )


def _run(inputs, trace=False, **kwargs):
    nc = _get_nc()
    params = _host_params(inputs)
    hs = np.asarray(inputs["hidden_states"], np.float32)
    in_maps = [dict(params, x=np.ascontiguousarray(hs[b])) for b in range(N_CORES)]
    try:
        res = run_bass_kernel_spmd(
            nc, in_maps, core_ids=list(range(N_CORES)), trace=trace, **kwargs
        )
    except ModuleNotFoundError:
        res = run_bass_kernel_spmd(
            nc, in_maps, core_ids=list(range(N_CORES)), trace=False, **kwargs
        )
    out = np.stack([res.results[b]["out"] for b in range(N_CORES)], axis=0)
    return out.astype(np.float32), res


def kernel(**inputs) -> np.ndarray:
    out, _ = _run(inputs, trace=False)
    return out


# revision 24
# speedup vs baseline: 469.1401x; 469.1401x over previous
"""Trainium2 Bass kernel for nn_CRFTModule (moe_routing).

Pure data parallel over batch: 8 cores, one batch row (4096 tokens) each.

Math per core (batch b, S=4096 tokens, H=1024):
  z      = gelu(x @ W1 + b1) @ W2 + b2              (critical-path detector)
  mask   = z > logit(0.7)                            (compare in logit space)
  aw     = softmax(x[last] @ sel_w + sel_b)          (adapter selector, 4-way)
  t      = gelu(x @ Dcat + db)                       (all 4 down-projs, [S,32])
  wm     = 0.3 * mask * (sum_a aw[a] (t_a @ up_w[a] + up_b[a]))
  out    = x + wm

Device pipeline (f16 matmuls, fp32 psum accumulation, fp32 residual):
  - x loaded naturally [tok, H] (one 2MB DMA per 512-token tile), cast to
    f16 (DVE), transposed on PE (128x128 blocks) into XT [H-part, tok].
  - detector mm1 runs transposed; mm2 runs back to NATURAL orientation
    (lhsT = gelu tile, rhs = W2 column) so the mask is a per-partition
    [tok,1] scalar.
  - adapter weights (softmax, per core) are folded into the up matrix once
    (U36w = U36 * wvec), the up matmul output is natural [tok, H], and the
    final op fuses mask-mult + residual-add in one DVE scalar_tensor_tensor.
  - softmax computed with the tanh identity so the whole kernel uses one
    ACT table set (gelu_and_others).
"""
import math

import numpy as np

import concourse.bacc as bacc
import concourse.mybir as mybir
from concourse.bass import ts
from concourse.tile import TileContext
from concourse.bass_utils import run_bass_kernel_spmd

dt = mybir.dt
AF = mybir.ActivationFunctionType
ALU = mybir.AluOpType

B, S, H = 8, 4096, 1024
A_DIM, N_ADAPT = 8, 4
PD = H // 2              # 512 detector hidden dim
SCALE = 0.3
THRESH = 0.7
N_CORES = 8
T = 512                  # tokens per tile
N_TILES = S // T         # 8
KUP = N_ADAPT * A_DIM + N_ADAPT  # 36

# f32 const blob column layout: b1(4) thr(1) selw(32) db(1) selb(1) o14(4) i4(4) e36(36)
_F32_COLS = 83
# f16 const blob column layout: w1(8*512) | dcat(8*32) | w2(4) | idh(128)
_F16_COLS = 8 * PD + 8 * 32 + 4 + 128


def _build():
    nc = bacc.Bacc("TRN2", target_bir_lowering=False, debug=False)

    x = nc.declare_dram_parameter("x", [S, H], dt.float32, isOutput=False)
    fb16 = nc.declare_dram_parameter("fb16", [128, _F16_COLS], dt.float16, isOutput=False)
    fb32 = nc.declare_dram_parameter("fb32", [128, _F32_COLS], dt.float32, isOutput=False)
    u36 = nc.declare_dram_parameter("u36", [KUP, H], dt.float16, isOutput=False)
    out = nc.declare_dram_parameter("out", [S, H], dt.float32, isOutput=True)

    with TileContext(nc) as tc:
        with (
            tc.tile_pool(name="consts", bufs=1) as cp,
            tc.tile_pool(name="work", bufs=2) as wp,
            tc.tile_pool(name="psum", bufs=2, space="PSUM") as pp,
        ):
            # prefetch tile 0 activations (two halves) before const loads
            X0 = wp.tile([128, 4, H], dt.float32, tag="X", name="Xpre", bufs=4)
            for h in range(2):
                nc.sync.dma_start(
                    out=X0[:, 2 * h : 2 * h + 2, :],
                    in_=x[h * 256 : (h + 1) * 256, :].rearrange(
                        "(j p) h -> p j h", p=128
                    ),
                )

            # ---- constants ----
            # idh (identity) first: transposes need it before the big blob lands
            c16 = cp.tile([128, _F16_COLS], dt.float16, tag="c16")
            _ID0 = 8 * PD + 260
            nc.sync.dma_start(out=c16[:, _ID0:_F16_COLS], in_=fb16[:, _ID0:_F16_COLS])
            nc.sync.dma_start(out=c16[:, 0:_ID0], in_=fb16[:, 0:_ID0])
            c32 = cp.tile([128, _F32_COLS], dt.float32, tag="c32")
            nc.sync.dma_start(out=c32[:], in_=fb32[:])
            u_sb = cp.tile([KUP, H], dt.float16, tag="u36")
            nc.sync.dma_start(out=u_sb[:], in_=u36[:])
            xlast0 = cp.tile([128, 8], dt.float32, tag="xlast")
            nc.sync.dma_start(
                out=xlast0[:],
                in_=x.rearrange("s (c p) -> p s c", p=128)[:, S - 1, :],
            )

            w1v = c16[:, 0 : 8 * PD].rearrange("p (c n) -> p c n", c=8)
            dcv = c16[:, 8 * PD : 8 * PD + 256].rearrange("p (c n) -> p c n", c=8)
            w2v = c16[:, 8 * PD + 256 : 8 * PD + 260]
            idh = c16[:, 8 * PD + 260 : 8 * PD + 388]
            b1v = c32[:, 0:4]
            thrv = c32[:, 4:5]
            selwv = c32[:, 5:37].rearrange("p (c a) -> p c a", c=8)
            dbv = c32[0:32, 37:38]
            selbv = c32[0:4, 38:39]
            o14v = c32[0:1, 39:43]
            i4v = c32[0:4, 43:47]
            e36v = c32[0:4, 47:83]

            # dummy ACT op so the gelu table set loads during startup DMAs
            dummy = cp.tile([1, 1], dt.float32, tag="dummy")
            nc.scalar.copy(dummy[:], c32[0:1, 0:1])

            # ---- adapter selector (once per core) ----
            xlast = xlast0
            ps_sel = pp.tile([4, 1], dt.float32, tag="small", bufs=1)
            for c in range(8):
                nc.tensor.matmul(
                    ps_sel[:], selwv[:, c, :], xlast[:, c : c + 1],
                    start=(c == 0), stop=(c == 7),
                )
            # t = tanh((z + sel_b)/2)  -> exp(z+sel_b) = (1+t)/(1-t)
            t4 = cp.tile([4, 1], dt.float32, tag="t4")
            nc.scalar.activation(t4[:], ps_sel[:], AF.Tanh, bias=selbv, scale=0.5)
            num4 = cp.tile([4, 1], dt.float32, tag="num4")
            nc.vector.tensor_scalar(num4[:], t4[:], 1.0, None, ALU.add)
            den4 = cp.tile([4, 1], dt.float32, tag="den4")
            nc.vector.tensor_scalar(den4[:], t4[:], -1.0, 1.0, ALU.mult, ALU.add)
            rden4 = cp.tile([4, 1], dt.float32, tag="rden4")
            nc.vector.reciprocal(rden4[:], den4[:])
            e4 = cp.tile([4, 1], dt.float32, tag="e4")
            nc.vector.tensor_mul(e4[:], num4[:], rden4[:])
            ps_et = pp.tile([1, 4], dt.float32, tag="small", bufs=1)
            nc.tensor.matmul(ps_et[:], e4[:], i4v, start=True, stop=True)
            ssum = cp.tile([1, 1], dt.float32, tag="ssum")
            nc.vector.reduce_sum(ssum[:], ps_et[:], axis=mybir.AxisListType.X)
            rsum = cp.tile([1, 1], dt.float32, tag="rsum")
            nc.vector.reciprocal(rsum[:], ssum[:])
            ps_rs = pp.tile([4, 1], dt.float32, tag="small", bufs=1)
            nc.tensor.matmul(ps_rs[:], o14v, rsum[:], start=True, stop=True)
            w4 = cp.tile([4, 1], dt.float32, tag="w4")
            nc.vector.tensor_tensor(w4[:], e4[:], ps_rs[:], ALU.mult)
            ps_wv = pp.tile([KUP, 1], dt.float32, tag="small", bufs=1)
            nc.tensor.matmul(ps_wv[:], e36v, w4[:], start=True, stop=True)
            wv_sb = cp.tile([KUP, 1], dt.float32, tag="wv")
            nc.scalar.copy(wv_sb[:], ps_wv[:])
            # fold adapter weights into the up matrix, once per core
            uw_sb = cp.tile([KUP, H], dt.float16, tag="uw")
            nc.vector.tensor_scalar(uw_sb[:], u_sb[:], wv_sb[:], None, ALU.mult)

            # ---- main loop over token tiles ----
            for i in range(N_TILES):
                if i == 0:
                    Xp = X0
                else:
                    Xp = wp.tile([128, 4, H], dt.float32, tag="X", name=f"X{i}", bufs=4)
                    nc.sync.dma_start(
                        out=Xp[:],
                        in_=x[i * T : (i + 1) * T, :].rearrange(
                            "(j p) h -> p j h", p=128
                        ),
                    )

                Xh = [
                    wp.tile([128, 2, H], dt.float16, tag="Xh", name=f"Xh{i}_{h}", bufs=4)
                    for h in range(2)
                ]
                for h in range(2):
                    nc.vector.tensor_copy(Xh[h][:], Xp[:, 2 * h : 2 * h + 2, :])

                # transpose x -> XT[q][:, dc, :] (chunk c = 2q+dc), packed psum
                XT = []
                for q in range(4):
                    ps_xt = pp.tile([128, 2, T], dt.float16, tag="xt", name=f"psxt{i}_{q}")
                    for dc in range(2):
                        c = 2 * q + dc
                        for j in range(4):
                            nc.tensor.transpose(
                                ps_xt[:, dc, ts(j, 128)],
                                Xh[j // 2][:, j % 2, ts(c, 128)],
                                idh,
                            )
                    xt = wp.tile([128, 2, T], dt.float16, tag="XT", name=f"XT{i}_{q}", bufs=8)
                    nc.scalar.copy(xt[:], ps_xt[:])
                    XT.append(xt)

                def xtc(c):
                    return XT[c // 2][:, c % 2, :]

                # down-proj + gelu; rows 32:36 of G are ones (up-proj bias)
                ps_t = pp.tile([32, T], dt.float32, tag="small", name=f"pst{i}", bufs=1)
                for c in range(8):
                    nc.tensor.matmul(
                        ps_t[:], dcv[:, c, :], xtc(c),
                        start=(c == 0), stop=(c == 7),
                    )
                G = wp.tile([KUP, T], dt.float16, tag="G", name=f"G{i}", bufs=2)
                nc.gpsimd.memset(G[32:KUP, :], 1.0)
                nc.scalar.activation(G[0:32, :], ps_t[:], AF.Gelu, bias=dbv)

                # detector mm1 + gelu
                Hs = []
                for m in range(4):
                    ps_h = pp.tile([128, T], dt.float32, tag="h", name=f"psh{i}_{m}")
                    for c in range(8):
                        nc.tensor.matmul(
                            ps_h[:], w1v[:, c, ts(m, 128)], xtc(c),
                            start=(c == 0), stop=(c == 7),
                        )
                    hm = wp.tile([128, T], dt.float16, tag="Hs", name=f"Hs{i}_{m}", bufs=5)
                    nc.scalar.activation(
                        hm[:], ps_h[:], AF.Gelu, bias=b1v[:, m : m + 1]
                    )
                    Hs.append(hm)

                # detector mm2, natural orientation: z[tok,1] per token chunk j
                ps_z = pp.tile([128, 4], dt.float32, tag="small", name=f"psz{i}", bufs=1)
                for j in range(4):
                    for m in range(4):
                        nc.tensor.matmul(
                            ps_z[:, j : j + 1], Hs[m][:, ts(j, 128)],
                            w2v[:, m : m + 1],
                            start=(m == 0), stop=(m == 3),
                        )
                maskn = wp.tile([128, 4], dt.float32, tag="maskn", name=f"maskn{i}", bufs=2)
                nc.vector.tensor_scalar(maskn[:], ps_z[:], thrv, None, ALU.is_gt)

                # up-proj (natural layout) + fused mask*psum + residual + store
                for j in range(4):
                    for n in range(2):
                        ps_w = pp.tile(
                            [128, PD], dt.float32, tag="w", name=f"psw{i}_{j}_{n}", bufs=3
                        )
                        nc.tensor.matmul(
                            ps_w[:], G[:, ts(j, 128)], uw_sb[:, ts(n, PD)],
                            start=True, stop=True,
                        )
                        nc.vector.scalar_tensor_tensor(
                            Xp[:, j, ts(n, PD)], ps_w[:], maskn[:, j : j + 1],
                            Xp[:, j, ts(n, PD)], ALU.mult, ALU.add,
                        )
                    if i == N_TILES - 1:
                        # last tile: store per 128-token chunk to shorten the tail
                        nc.gpsimd.dma_start(
                            out=out[i * T + j * 128 : i * T + (j + 1) * 128, :],
                            in_=Xp[:, j, :],
                        )
                    elif j % 2 == 1:
                        h = j // 2
                        nc.gpsimd.dma_start(
                            out=out[
                                i * T + h * 256 : i * T + (h + 1) * 256, :
                            ].rearrange("(j p) h -> p j h", p=128),
                            in_=Xp[:, 2 * h : 2 * h + 2, :],
                        )

    nc.compile()
    return nc


_CACHE = {}


def _get_nc():
    if "nc" not in _CACHE:
        _CACHE["nc"] = _build()
    return _CACHE["nc"]


def _host_params(inputs):
    f32 = np.float32
    f16 = np.float16
    pd_w1 = np.asarray(inputs["pd_w1"], f32)          # [H, PD]
    pd_b1 = np.asarray(inputs["pd_b1"], f32)          # [PD]
    pd_w2 = np.asarray(inputs["pd_w2"], f32)          # [PD, 1]
    pd_b2 = np.asarray(inputs["pd_b2"], f32)          # [1]
    down_w = np.asarray(inputs["down_w"], f32)        # [A, H, d]
    down_b = np.asarray(inputs["down_b"], f32)        # [A, d]
    up_w = np.asarray(inputs["up_w"], f32)            # [A, d, H]
    up_b = np.asarray(inputs["up_b"], f32)            # [A, H]
    sel_w = np.asarray(inputs["sel_w"], f32)          # [H, A]
    sel_b = np.asarray(inputs["sel_b"], f32)          # [A]

    # f16 blob: w1 | dcat | w2 | idh
    w1s = pd_w1.reshape(8, 128, PD).transpose(1, 0, 2).reshape(128, 8 * PD)
    dcat = down_w.transpose(1, 0, 2).reshape(H, 32)
    dcats = dcat.reshape(8, 128, 32).transpose(1, 0, 2).reshape(128, 256)
    w2s = pd_w2.reshape(4, 128).T
    fb16 = np.concatenate([w1s, dcats, w2s, np.eye(128)], axis=1).astype(f16)
    assert fb16.shape == (128, _F16_COLS)

    # f32 blob: b1(4) | thr(1) | selw(32) | db(1) | selb(1) | o14(4) | i4(4) | e36(36)
    b1s = pd_b1.reshape(4, 128).T
    thr = np.full((128, 1), math.log(THRESH / (1.0 - THRESH)) - float(pd_b2[0]), f32)
    selws = sel_w.reshape(8, 128, 4).transpose(1, 0, 2).reshape(128, 32)
    dbcol = np.zeros((128, 1), f32)
    dbcol[0:32, 0] = down_b.reshape(32)
    selbcol = np.zeros((128, 1), f32)
    selbcol[0:4, 0] = sel_b / 2.0
    o14 = np.zeros((128, 4), f32)
    o14[0, :] = 1.0
    i4m = np.zeros((128, 4), f32)
    i4m[0:4, :] = np.eye(4)
    e36m = np.zeros((128, KUP), f32)
    for r in range(32):
        e36m[r // 8, r] = 1.0
    for a in range(4):
        e36m[a, 32 + a] = 1.0
    fb32 = np.concatenate(
        [b1s, thr, selws, dbcol, selbcol, o14, i4m, e36m], axis=1
    ).astype(f32)
    assert fb32.shape == (128, _F32_COLS)

    u36 = np.concatenate(
        [SCALE * up_w.reshape(32, H), SCALE * up_b], axis=0
    ).astype(f16)
    return dict(fb16=fb16, fb32=fb32, u36=u36)


def _run(inputs, trace=False, **kwargs):
    nc = _get_nc()
    params = _host_params(inputs)
    hs = np.asarray(inputs["hidden_states"], np.float32)
    in_maps = [dict(params, x=np.ascontiguousarray(hs[b])) for b in range(N_CORES)]
    try:
        res = run_bass_kernel_spmd(
            nc, in_maps, core_ids=list(range(N_CORES)), trace=trace, **kwargs
        )
    except ModuleNotFoundError:
        res = run_bass_kernel_spmd(
            nc, in_maps, core_ids=list(range(N_CORES)), trace=False, **kwargs
        )
    out = np.stack([res.results[b]["out"] for b in range(N_CORES)], axis=0)
    return out.astype(np.float32), res


def kernel(**inputs) -> np.ndarray:
    out, _ = _run(inputs, trace=False)
    return out


# revision 25
# speedup vs baseline: 470.4398x; 1.0028x over previous
"""Trainium2 Bass kernel for nn_CRFTModule (moe_routing).

Pure data parallel over batch: 8 cores, one batch row (4096 tokens) each.

Math per core (batch b, S=4096 tokens, H=1024):
  z      = gelu(x @ W1 + b1) @ W2 + b2              (critical-path detector)
  mask   = z > logit(0.7)                            (compare in logit space)
  aw     = softmax(x[last] @ sel_w + sel_b)          (adapter selector, 4-way)
  t      = gelu(x @ Dcat + db)                       (all 4 down-projs, [S,32])
  wm     = 0.3 * mask * (sum_a aw[a] (t_a @ up_w[a] + up_b[a]))
  out    = x + wm

Device pipeline (f16 matmuls, fp32 psum accumulation, fp32 residual):
  - x loaded naturally [tok, H] (one 2MB DMA per 512-token tile), cast to
    f16 (DVE), transposed on PE (128x128 blocks) into XT [H-part, tok].
  - detector mm1 runs transposed; mm2 runs back to NATURAL orientation
    (lhsT = gelu tile, rhs = W2 column) so the mask is a per-partition
    [tok,1] scalar.
  - adapter weights (softmax, per core) are folded into the up matrix once
    (U36w = U36 * wvec), the up matmul output is natural [tok, H], and the
    final op fuses mask-mult + residual-add in one DVE scalar_tensor_tensor.
  - softmax computed with the tanh identity so the whole kernel uses one
    ACT table set (gelu_and_others).
"""
import math

import numpy as np

import concourse.bacc as bacc
import concourse.mybir as mybir
from concourse.bass import ts
from concourse.tile import TileContext
from concourse.bass_utils import run_bass_kernel_spmd

dt = mybir.dt
AF = mybir.ActivationFunctionType
ALU = mybir.AluOpType

B, S, H = 8, 4096, 1024
A_DIM, N_ADAPT = 8, 4
PD = H // 2              # 512 detector hidden dim
SCALE = 0.3
THRESH = 0.7
N_CORES = 8
T = 512                  # tokens per tile
N_TILES = S // T         # 8
KUP = N_ADAPT * A_DIM + N_ADAPT  # 36

# f32 const blob column layout: b1(4) thr(1) selw(32) db(1) selb(1) o14(4) i4(4) e36(36)
_F32_COLS = 83
# f16 const blob column layout: w1(8*512) | dcat(8*32) | w2(4) | idh(128)
_F16_COLS = 8 * PD + 8 * 32 + 4 + 128


def _build():
    nc = bacc.Bacc("TRN2", target_bir_lowering=False, debug=False)

    x = nc.declare_dram_parameter("x", [S, H], dt.float32, isOutput=False)
    fb16 = nc.declare_dram_parameter("fb16", [128, _F16_COLS], dt.float16, isOutput=False)
    fb32 = nc.declare_dram_parameter("fb32", [128, _F32_COLS], dt.float32, isOutput=False)
    u36 = nc.declare_dram_parameter("u36", [KUP, H], dt.float16, isOutput=False)
    out = nc.declare_dram_parameter("out", [S, H], dt.float32, isOutput=True)

    with TileContext(nc) as tc:
        with (
            tc.tile_pool(name="consts", bufs=1) as cp,
            tc.tile_pool(name="work", bufs=2) as wp,
            tc.tile_pool(name="psum", bufs=2, space="PSUM") as pp,
        ):
            # prefetch tile 0 activations (two halves) before const loads
            X0 = wp.tile([128, 4, H], dt.float32, tag="X", name="Xpre", bufs=4)
            for h in range(2):
                nc.sync.dma_start(
                    out=X0[:, 2 * h : 2 * h + 2, :],
                    in_=x[h * 256 : (h + 1) * 256, :].rearrange(
                        "(j p) h -> p j h", p=128
                    ),
                )

            # ---- constants ----
            # idh (identity) first: transposes need it before the big blob lands
            c16 = cp.tile([128, _F16_COLS], dt.float16, tag="c16")
            _ID0 = 8 * PD + 260
            nc.sync.dma_start(out=c16[:, _ID0:_F16_COLS], in_=fb16[:, _ID0:_F16_COLS])
            nc.sync.dma_start(out=c16[:, 0:_ID0], in_=fb16[:, 0:_ID0])
            c32 = cp.tile([128, _F32_COLS], dt.float32, tag="c32")
            nc.sync.dma_start(out=c32[:], in_=fb32[:])
            u_sb = cp.tile([KUP, H], dt.float16, tag="u36")
            nc.sync.dma_start(out=u_sb[:], in_=u36[:])
            xlast0 = cp.tile([128, 8], dt.float32, tag="xlast")
            nc.sync.dma_start(
                out=xlast0[:],
                in_=x.rearrange("s (c p) -> p s c", p=128)[:, S - 1, :],
            )

            w1v = c16[:, 0 : 8 * PD].rearrange("p (c n) -> p c n", c=8)
            dcv = c16[:, 8 * PD : 8 * PD + 256].rearrange("p (c n) -> p c n", c=8)
            w2v = c16[:, 8 * PD + 256 : 8 * PD + 260]
            idh = c16[:, 8 * PD + 260 : 8 * PD + 388]
            b1v = c32[:, 0:4]
            thrv = c32[:, 4:5]
            selwv = c32[:, 5:37].rearrange("p (c a) -> p c a", c=8)
            dbv = c32[0:32, 37:38]
            selbv = c32[0:4, 38:39]
            o14v = c32[0:1, 39:43]
            i4v = c32[0:4, 43:47]
            e36v = c32[0:4, 47:83]

            # dummy ACT op so the gelu table set loads during startup DMAs
            dummy = cp.tile([1, 1], dt.float32, tag="dummy")
            nc.scalar.copy(dummy[:], c32[0:1, 0:1])

            # ---- adapter selector (once per core) ----
            xlast = xlast0
            ps_sel = pp.tile([4, 1], dt.float32, tag="small", bufs=1)
            for c in range(8):
                nc.tensor.matmul(
                    ps_sel[:], selwv[:, c, :], xlast[:, c : c + 1],
                    start=(c == 0), stop=(c == 7),
                )
            # t = tanh((z + sel_b)/2)  -> exp(z+sel_b) = (1+t)/(1-t)
            t4 = cp.tile([4, 1], dt.float32, tag="t4")
            nc.scalar.activation(t4[:], ps_sel[:], AF.Tanh, bias=selbv, scale=0.5)
            num4 = cp.tile([4, 1], dt.float32, tag="num4")
            nc.vector.tensor_scalar(num4[:], t4[:], 1.0, None, ALU.add)
            den4 = cp.tile([4, 1], dt.float32, tag="den4")
            nc.vector.tensor_scalar(den4[:], t4[:], -1.0, 1.0, ALU.mult, ALU.add)
            rden4 = cp.tile([4, 1], dt.float32, tag="rden4")
            nc.vector.reciprocal(rden4[:], den4[:])
            e4 = cp.tile([4, 1], dt.float32, tag="e4")
            nc.vector.tensor_mul(e4[:], num4[:], rden4[:])
            ps_et = pp.tile([1, 4], dt.float32, tag="small", bufs=1)
            nc.tensor.matmul(ps_et[:], e4[:], i4v, start=True, stop=True)
            ssum = cp.tile([1, 1], dt.float32, tag="ssum")
            nc.vector.reduce_sum(ssum[:], ps_et[:], axis=mybir.AxisListType.X)
            rsum = cp.tile([1, 1], dt.float32, tag="rsum")
            nc.vector.reciprocal(rsum[:], ssum[:])
            ps_rs = pp.tile([4, 1], dt.float32, tag="small", bufs=1)
            nc.tensor.matmul(ps_rs[:], o14v, rsum[:], start=True, stop=True)
            w4 = cp.tile([4, 1], dt.float32, tag="w4")
            nc.vector.tensor_tensor(w4[:], e4[:], ps_rs[:], ALU.mult)
            ps_wv = pp.tile([KUP, 1], dt.float32, tag="small", bufs=1)
            nc.tensor.matmul(ps_wv[:], e36v, w4[:], start=True, stop=True)
            wv_sb = cp.tile([KUP, 1], dt.float32, tag="wv")
            nc.scalar.copy(wv_sb[:], ps_wv[:])
            # fold adapter weights into the up matrix, once per core
            uw_sb = cp.tile([KUP, H], dt.float16, tag="uw")
            nc.vector.tensor_scalar(uw_sb[:], u_sb[:], wv_sb[:], None, ALU.mult)

            # ---- main loop over token tiles ----
            for i in range(N_TILES):
                if i == 0:
                    Xp = X0
                else:
                    Xp = wp.tile([128, 4, H], dt.float32, tag="X", name=f"X{i}", bufs=4)
                    nc.sync.dma_start(
                        out=Xp[:],
                        in_=x[i * T : (i + 1) * T, :].rearrange(
                            "(j p) h -> p j h", p=128
                        ),
                    )

                Xh = [
                    wp.tile([128, 2, H], dt.float16, tag="Xh", name=f"Xh{i}_{h}", bufs=4)
                    for h in range(2)
                ]
                for h in range(2):
                    nc.vector.tensor_copy(Xh[h][:], Xp[:, 2 * h : 2 * h + 2, :])

                # transpose x -> XT[q][:, dc, :] (chunk c = 2q+dc), packed psum
                XT = []
                for q in range(4):
                    ps_xt = pp.tile([128, 2, T], dt.float16, tag="xt", name=f"psxt{i}_{q}")
                    for dc in range(2):
                        c = 2 * q + dc
                        for j in range(4):
                            nc.tensor.transpose(
                                ps_xt[:, dc, ts(j, 128)],
                                Xh[j // 2][:, j % 2, ts(c, 128)],
                                idh,
                            )
                    xt = wp.tile([128, 2, T], dt.float16, tag="XT", name=f"XT{i}_{q}", bufs=8)
                    nc.scalar.copy(xt[:], ps_xt[:])
                    XT.append(xt)

                def xtc(c):
                    return XT[c // 2][:, c % 2, :]

                # down-proj + gelu; rows 32:36 of G are ones (up-proj bias)
                ps_t = pp.tile([32, T], dt.float32, tag="small", name=f"pst{i}", bufs=1)
                for c in range(8):
                    nc.tensor.matmul(
                        ps_t[:], dcv[:, c, :], xtc(c),
                        start=(c == 0), stop=(c == 7),
                    )
                G = wp.tile([KUP, T], dt.float16, tag="G", name=f"G{i}", bufs=2)
                nc.gpsimd.memset(G[32:KUP, :], 1.0)
                nc.scalar.activation(G[0:32, :], ps_t[:], AF.Gelu, bias=dbv)

                # detector mm1 + gelu
                Hs = []
                for m in range(4):
                    ps_h = pp.tile([128, T], dt.float32, tag="h", name=f"psh{i}_{m}")
                    for c in range(8):
                        nc.tensor.matmul(
                            ps_h[:], w1v[:, c, ts(m, 128)], xtc(c),
                            start=(c == 0), stop=(c == 7),
                        )
                    hm = wp.tile([128, T], dt.float16, tag="Hs", name=f"Hs{i}_{m}", bufs=5)
                    nc.scalar.activation(
                        hm[:], ps_h[:], AF.Gelu, bias=b1v[:, m : m + 1]
                    )
                    Hs.append(hm)

                # detector mm2, natural orientation: z[tok,1] per token chunk j
                ps_z = pp.tile([128, 4], dt.float32, tag="small", name=f"psz{i}", bufs=1)
                for j in range(4):
                    for m in range(4):
                        nc.tensor.matmul(
                            ps_z[:, j : j + 1], Hs[m][:, ts(j, 128)],
                            w2v[:, m : m + 1],
                            start=(m == 0), stop=(m == 3),
                        )
                maskn = wp.tile([128, 4], dt.float32, tag="maskn", name=f"maskn{i}", bufs=2)
                nc.vector.tensor_scalar(maskn[:], ps_z[:], thrv, None, ALU.is_gt)

                # up-proj (natural layout) + fused mask*psum + residual + store
                for j in range(4):
                    for n in range(2):
                        ps_w = pp.tile(
                            [128, PD], dt.float32, tag="w", name=f"psw{i}_{j}_{n}", bufs=3
                        )
                        nc.tensor.matmul(
                            ps_w[:], G[:, ts(j, 128)], uw_sb[:, ts(n, PD)],
                            start=True, stop=True,
                        )
                        nc.vector.scalar_tensor_tensor(
                            Xp[:, j, ts(n, PD)], ps_w[:], maskn[:, j : j + 1],
                            Xp[:, j, ts(n, PD)], ALU.mult, ALU.add,
                        )
                    if i == N_TILES - 1:
                        # last tile: store per 128-token chunk via HWDGE (short tail)
                        nc.sync.dma_start(
                            out=out[i * T + j * 128 : i * T + (j + 1) * 128, :],
                            in_=Xp[:, j, :],
                        )
                    elif j % 2 == 1:
                        h = j // 2
                        nc.gpsimd.dma_start(
                            out=out[
                                i * T + h * 256 : i * T + (h + 1) * 256, :
                            ].rearrange("(j p) h -> p j h", p=128),
                            in_=Xp[:, 2 * h : 2 * h + 2, :],
                        )

    nc.compile()
    return nc


_CACHE = {}


def _get_nc():
    if "nc" not in _CACHE:
        _CACHE["nc"] = _build()
    return _CACHE["nc"]


def _host_params(inputs):
    f32 = np.float32
    f16 = np.float16
    pd_w1 = np.asarray(inputs["pd_w1"], f32)          # [H, PD]
    pd_b1 = np.asarray(inputs["pd_b1"], f32)          # [PD]
    pd_w2 = np.asarray(inputs["pd_w2"], f32)          # [PD, 1]
    pd_b2 = np.asarray(inputs["pd_b2"], f32)          # [1]
    down_w = np.asarray(inputs["down_w"], f32)        # [A, H, d]
    down_b = np.asarray(inputs["down_b"], f32)        # [A, d]
    up_w = np.asarray(inputs["up_w"], f32)            # [A, d, H]
    up_b = np.asarray(inputs["up_b"], f32)            # [A, H]
    sel_w = np.asarray(inputs["sel_w"], f32)          # [H, A]
    sel_b = np.asarray(inputs["sel_b"], f32)          # [A]

    # f16 blob: w1 | dcat | w2 | idh
    w1s = pd_w1.reshape(8, 128, PD).transpose(1, 0, 2).reshape(128, 8 * PD)
    dcat = down_w.transpose(1, 0, 2).reshape(H, 32)
    dcats = dcat.reshape(8, 128, 32).transpose(1, 0, 2).reshape(128, 256)
    w2s = pd_w2.reshape(4, 128).T
    fb16 = np.concatenate([w1s, dcats, w2s, np.eye(128)], axis=1).astype(f16)
    assert fb16.shape == (128, _F16_COLS)

    # f32 blob: b1(4) | thr(1) | selw(32) | db(1) | selb(1) | o14(4) | i4(4) | e36(36)
    b1s = pd_b1.reshape(4, 128).T
    thr = np.full((128, 1), math.log(THRESH / (1.0 - THRESH)) - float(pd_b2[0]), f32)
    selws = sel_w.reshape(8, 128, 4).transpose(1, 0, 2).reshape(128, 32)
    dbcol = np.zeros((128, 1), f32)
    dbcol[0:32, 0] = down_b.reshape(32)
    selbcol = np.zeros((128, 1), f32)
    selbcol[0:4, 0] = sel_b / 2.0
    o14 = np.zeros((128, 4), f32)
    o14[0, :] = 1.0
    i4m = np.zeros((128, 4), f32)
    i4m[0:4, :] = np.eye(4)
    e36m = np.zeros((128, KUP), f32)
    for r in range(32):
        e36m[r // 8, r] = 1.0
    for a in range(4):
        e36m[a, 32 + a] = 1.0
    fb32 = np.concatenate(
        [b1s, thr, selws, dbcol, selbcol, o14, i4m, e36m], axis=1
    ).astype(f32)
    assert fb32.shape == (128, _F32_COLS)

    u36 = np.concatenate(
        [SCALE * up_w.reshape(32, H), SCALE * up_b], axis=0
    ).astype(f16)
    return dict(fb16=fb16, fb32=fb32, u36=u36)


def _run(inputs, trace=False, **kwargs):
    nc = _get_nc()
    params = _host_params(inputs)
    hs = np.asarray(inputs["hidden_states"], np.float32)
    in_maps = [dict(params, x=np.ascontiguousarray(hs[b])) for b in range(N_CORES)]
    try:
        res = run_bass_kernel_spmd(
            nc, in_maps, core_ids=list(range(N_CORES)), trace=trace, **kwargs
        )
    except ModuleNotFoundError:
        res = run_bass_kernel_spmd(
            nc, in_maps, core_ids=list(range(N_CORES)), trace=False, **kwargs
        )
    out = np.stack([res.results[b]["out"] for b in range(N_CORES)], axis=0)
    return out.astype(np.float32), res


def kernel(**inputs) -> np.ndarray:
    out, _ = _run(inputs, trace=False)
    return out
